# revision 1
# baseline (speedup 1.0000x reference)
"""MeshGNN Trainium2 kernel.

Mathematical reduction: the reference broadcasts the text projection to all 12
mesh vertices, and the row-normalized kNN adjacency has identical row sums
(every vertex has exactly K_NN=6 neighbors), so node features remain identical
across vertices through every GNN layer.  The whole network therefore
collapses to a per-row MLP:

    h   = relu(x @ W0c + b0c)          W0c = W_text @ (s*W_gnn[0])  (384,256)
    h   = relu(h @ (s*W_gnn[l]) + b_gnn[l])   l = 1..3
    o36 = h @ W4c + b4c                W4c = tile(W_out, 12) (256,36)
    out = o36.reshape(B, 12, 3)        b4c = tile(b_out,12) + template.flat

where s = 6/(6+1e-6) is the common adjacency row sum.

Device strategy (8 cores, pure data parallel over the batch):
  - host pre-transposes each core's x shard to (384, 4096) so features sit on
    SBUF partitions; all matmuls then run in feature-on-partition layout with
    weights as the stationary operand and activations as the moving operand.
  - float32r (default) or bf16 matmuls, 1 PE cycle/row at N=512.
  - relu+bias fused into one op per layer (both m-tiles at once), alternating
    ScalarE / VectorE between layers to balance the two engines.
  - output computed as (36, 4096) on device; host transposes back.
"""

import os

import numpy as np

# ---------------------------------------------------------------- constants
B = 32768
CORES = 8
ROWS = B // CORES            # 4096 rows per core
TD = 384                     # text dim
H = 256                      # hidden
OUT = 36                     # 12 verts * 3 coords
NBLK = 8                     # row blocks per core
N = ROWS // NBLK             # 512 rows per block
KT0 = TD // 128              # 3 k-tiles for layer 0
KTH = H // 128               # 2 k-tiles for hidden layers
MT = H // 128                # 2 m-tiles for hidden outputs
GRP = 4                      # blocks interleaved per scheduling group

MM_DTYPE = os.environ.get("MESHGNN_DTYPE", "f32r")   # "f32r" | "bf16" | "fp8"

_BUILT = {}                  # cache: compiled Bass modules keyed by config


def _np_mm_dtype():
    if MM_DTYPE == "bf16":
        import ml_dtypes
        return ml_dtypes.bfloat16
    if MM_DTYPE == "fp8":
        import concourse.mybir as mybir
        return mybir.dt.np(mybir.dt.float8e4)
    return np.float32


def _build_bass(repeat=1, fake_relu=False, loop_repeat=0, zero_bias=None):
    """Build + compile the per-core Bass program (same NEFF on all cores).

    repeat > 1 re-runs the whole pipeline that many times inside one NEFF
    (identical outputs each pass) -- used for dispatch-free HW timing.
    loop_repeat > 0 wraps the pipeline in a device-side For_i loop executed
    that many times (identical outputs; ~2us barrier per back-edge) -- used
    for timing with enough device work to swamp dispatch noise entirely.
    fake_relu=True makes relu read a constant SBUF tile instead of PSUM
    (wrong results; PE never waits on ACT/DVE) -- PE-floor timing only.
    """
    import concourse.mybir as mybir
    import concourse.tile as tile
    from concourse import bacc

    if zero_bias is None:
        zero_bias = _BUILT.get("zero_bias", False)

    f32 = mybir.dt.float32
    is_fp8 = MM_DTYPE == "fp8"
    if is_fp8:
        mmdt = mybir.dt.float8e4
    elif MM_DTYPE == "bf16":
        mmdt = mybir.dt.bfloat16
    else:
        mmdt = mybir.dt.float32r
    grp = GRP if MM_DTYPE == "fp8" else 2
    DR = mybir.MatmulPerfMode.DoubleRow
    RELU = mybir.ActivationFunctionType.Relu
    IDENT = mybir.ActivationFunctionType.Identity
    ADD = mybir.AluOpType.add
    MAX = mybir.AluOpType.max

    nc = bacc.Bacc(
        "TRN2",
        target_bir_lowering=False,
        debug=False,
        enable_asserts=False,
        num_devices=CORES,
    )

    xt_d = nc.dram_tensor("xt", (TD, ROWS), mmdt, kind="ExternalInput")
    w0_d = nc.dram_tensor("w0", (TD, H), mmdt, kind="ExternalInput")
    wl_d = [
        nc.dram_tensor(f"w{l}", (H, H), mmdt, kind="ExternalInput")
        for l in (1, 2, 3)
    ]
    w4_d = nc.dram_tensor("w4", (H, OUT), mmdt, kind="ExternalInput")
    bl_d = None if zero_bias else [
        nc.dram_tensor(f"b{l}", (128, MT), f32, kind="ExternalInput")
        for l in (0, 1, 2, 3)
    ]
    b4_d = nc.dram_tensor("b4", (OUT, 1), f32, kind="ExternalInput")
    out_d = nc.dram_tensor("out", (OUT, ROWS), f32, kind="ExternalOutput")

    # x viewed as (partition, ktile, row): row-major (TD, ROWS) split over 128
    xt_v = xt_d.ap().rearrange("(k p) n -> p k n", p=128)

    with tile.TileContext(nc) as tc:
        with (
            tc.tile_pool(name="wp", bufs=1) as wp,
            tc.tile_pool(name="xp", bufs=3) as xp,
            tc.tile_pool(name="hp", bufs=3) as hp,
            tc.tile_pool(name="op", bufs=3) as op,
            tc.tile_pool(name="pp", bufs=6, space="PSUM") as pp,
            tc.tile_pool(name="pp4", bufs=2, space="PSUM") as pp4,
        ):
            # ---- weights / biases, loaded once
            # fp8 path: hidden layers use DoubleRow -- the stationary operand
            # is a 3D [128, 2, M] tile holding k-pairs (logical k = i*128+p),
            # gathered straight from the row-major DRAM weights by the DMA.
            w0_t, wl_t, w4_t = {}, {}, {}
            w0dr_t, w0k2_t, wldr_t = {}, {}, None
            if is_fp8:
                for m in range(MT):
                    ms = slice(m * 128, (m + 1) * 128)
                    t = wp.tile([128, 2, 144], mmdt, tag=f"w0dr_{m}")
                    nc.scalar.dma_start(
                        t[:, :, 0:128],
                        w0_d.ap()[0:256, ms].rearrange("(i p) m -> p i m", p=128),
                    )
                    w0dr_t[m] = t
                    t2 = wp.tile([128, 128], mmdt, tag=f"w0k2_{m}")
                    nc.scalar.dma_start(t2[:], w0_d.ap()[256:384, ms])
                    w0k2_t[m] = t2
                wldr_t = {}
                for li, l in enumerate((1, 2, 3)):
                    for m in range(MT):
                        ms = slice(m * 128, (m + 1) * 128)
                        t = wp.tile([128, 2, 144], mmdt, tag=f"w{l}dr_{m}")
                        nc.scalar.dma_start(
                            t[:, :, 0:128],
                            wl_d[li].ap()[:, ms].rearrange(
                                "(i p) m -> p i m", p=128
                            ),
                        )
                        wldr_t[l, m] = t
                w4dr = wp.tile([128, 2, 48], mmdt, tag="w4dr")
                nc.scalar.dma_start(
                    w4dr[:, :, 0:OUT], w4_d.ap().rearrange("(i p) m -> p i m", p=128)
                )
            else:
                for k in range(KT0):
                    for m in range(MT):
                        t = wp.tile([128, 128], mmdt, tag=f"w0_{k}_{m}")
                        nc.scalar.dma_start(
                            t[:],
                            w0_d.ap()[k * 128:(k + 1) * 128, m * 128:(m + 1) * 128],
                        )
                        w0_t[k, m] = t
                for li, l in enumerate((1, 2, 3)):
                    for k in range(KTH):
                        for m in range(MT):
                            t = wp.tile([128, 128], mmdt, tag=f"w{l}_{k}_{m}")
                            nc.scalar.dma_start(
                                t[:],
                                wl_d[li].ap()[
                                    k * 128:(k + 1) * 128, m * 128:(m + 1) * 128
                                ],
                            )
                            wl_t[l, k, m] = t
                for k in range(KTH):
                    t = wp.tile([128, OUT], mmdt, tag=f"w4_{k}")
                    nc.scalar.dma_start(t[:], w4_d.ap()[k * 128:(k + 1) * 128, :])
                    w4_t[k] = t
            bl_t = {}
            if not zero_bias:
                for l in range(4):
                    t = wp.tile([128, MT], f32, tag=f"b{l}")
                    nc.scalar.dma_start(t[:], bl_d[l].ap()[:])
                    bl_t[l] = t
            # bias broadcast to both m-tiles' column ranges for fused relu:
            # fused op covers (128, MT*N); bias AP must be per-partition, so
            # we keep per-m bias and slice the fused tile per m only for the
            # bias application -- i.e. still per-m ops. Instead we fuse by
            # applying relu over the 3D psum tile per m with one op each but
            # batching both m psum banks in one tile for scheduling locality.
            b4_t = wp.tile([OUT, 1], f32, tag="b4")
            nc.scalar.dma_start(b4_t[:], b4_d.ap()[:])

            # ---- main loop over repeats x pairs of 512-row blocks.
            # Two blocks are interleaved layer-by-layer so the PE always has
            # an independent matmul stream while the other block's relu
            # drains; per-k x DMAs let L0 start on the first k-tile.
            import contextlib

            loop_cm = (
                tc.For_i(0, loop_repeat, 1) if loop_repeat
                else contextlib.nullcontext()
            )
            with loop_cm:
                for rep in range(repeat):
                    for pair in range(NBLK // grp):
                            blks = tuple(range(grp * pair, grp * (pair + 1)))
                            xts = {}
                            for b in blks:
                                xt = xp.tile([128, KT0, N], mmdt, tag=f"x{b % grp}")
                                for k in range(KT0):
                                    nc.sync.dma_start(
                                        xt[:, k, :],
                                        xt_v[:, k, b * N:(b + 1) * N],
                                    )
                                xts[b] = xt

                            h_prev = {b: None for b in blks}
                            for l in range(4):
                                w_tiles = {} if is_fp8 else (
                                    w0_t if l == 0 else {
                                        (k, m): wl_t[l, k, m]
                                        for k in range(KTH) for m in range(MT)
                                    }
                                )
                                nk = KT0 if l == 0 else KTH
                                h_cur = {}
                                pss = {}
                                for b in blks:
                                    h_cur[b] = hp.tile(
                                        [128, MT, N], mmdt,
                                        name=f"hc{l}{b % grp}",
                                        tag=f"h{l}{b % grp}",
                                    )
                                    for m in range(MT):
                                        pss[b, m] = pp.tile(
                                            [128, N], f32, name="psb", tag="ps"
                                        )
                                for b in blks:
                                    for m in range(MT):
                                        ps = pss[b, m]
                                        if is_fp8 and l == 0:
                                            nc.tensor.matmul(
                                                ps[:], w0dr_t[m][:, :, 0:128],
                                                xts[b][:, 0:2, :],
                                                start=True, stop=False,
                                                perf_mode=DR,
                                            )
                                            nc.tensor.matmul(
                                                ps[:], w0k2_t[m][:],
                                                xts[b][:, 2, :],
                                                start=False, stop=True,
                                            )
                                        elif is_fp8:
                                            nc.tensor.matmul(
                                                ps[:],
                                                wldr_t[l, m][:, :, 0:128],
                                                h_prev[b][:, :, :],
                                                start=True, stop=True,
                                                perf_mode=DR,
                                            )
                                        else:
                                            for k in range(nk):
                                                rhs = (
                                                    xts[b][:, k, :] if l == 0
                                                    else h_prev[b][:, k, :]
                                                )
                                                nc.tensor.matmul(
                                                    ps[:],
                                                    w_tiles[k, m][:],
                                                    rhs,
                                                    start=(k == 0),
                                                    stop=(k == nk - 1),
                                                )
                                        # relu as soon as this m-group stops;
                                        # engines alternate for balance
                                        if zero_bias:
                                            if (l + b + m) % 2 == 0:
                                                nc.scalar.activation(
                                                    h_cur[b][:, m, :], ps[:],
                                                    RELU,
                                                )
                                            else:
                                                nc.vector.tensor_scalar(
                                                    h_cur[b][:, m, :], ps[:],
                                                    0.0, None, MAX,
                                                )
                                        else:
                                            if (l + b + m) % 2 == 0:
                                                nc.scalar.activation(
                                                    h_cur[b][:, m, :], ps[:],
                                                    RELU,
                                                    bias=bl_t[l][:, m:m + 1],
                                                )
                                            else:
                                                nc.vector.tensor_scalar(
                                                    h_cur[b][:, m, :], ps[:],
                                                    bl_t[l][:, m:m + 1], 0.0,
                                                    ADD, MAX,
                                                )
                                for b in blks:
                                    h_prev[b] = h_cur[b]

                            for b in blks:
                                ps4 = pp4.tile([OUT, N], f32, tag="ps4")
                                if is_fp8:
                                    nc.tensor.matmul(
                                        ps4[:], w4dr[:, :, 0:OUT], h_prev[b][:, :, :],
                                        start=True, stop=True, perf_mode=DR,
                                    )
                                else:
                                    for k in range(KTH):
                                        nc.tensor.matmul(
                                            ps4[:],
                                            w4_t[k][:],
                                            h_prev[b][:, k, :],
                                            start=(k == 0),
                                            stop=(k == KTH - 1),
                                        )
                                ob = op.tile([OUT, N], f32, tag="ob")
                                if b % 2 == 0:
                                    nc.scalar.activation(
                                        ob[:], ps4[:], IDENT, bias=b4_t[:]
                                    )
                                else:
                                    nc.vector.tensor_scalar(
                                        ob[:], ps4[:], b4_t[:], None, ADD,
                                    )
                                nc.sync.dma_start(
                                    out_d.ap()[:, b * N:(b + 1) * N], ob[:]
                                )

    nc.compile()
    return nc


def _fold_weights(W_text, b_text, W_gnn, b_gnn, W_out, b_out, adjacency, template):
    s_rows = adjacency.astype(np.float64).sum(axis=1)
    if np.ptp(s_rows) > 1e-5:
        raise ValueError("adjacency row sums are not uniform; collapse invalid")
    s = float(s_rows.mean())

    W0c = (W_text.astype(np.float64) @ (s * W_gnn[0].astype(np.float64)))
    b0c = s * (b_text.astype(np.float64) @ W_gnn[0].astype(np.float64)) + b_gnn[0]
    Wl = [s * W_gnn[l].astype(np.float64) for l in (1, 2, 3)]
    bl = [b_gnn[l] for l in (1, 2, 3)]
    W4c = np.tile(W_out, (1, 12))
    b4c = np.tile(b_out, 12) + template.reshape(36)

    mdt = _np_mm_dtype()

    def cvt(a, dt):
        return np.ascontiguousarray(np.asarray(a, dtype=np.float32).astype(dt))

    biases = [
        cvt(np.asarray(b, dtype=np.float64).reshape(MT, 128).T, np.float32)
        for b in [b0c, *bl]
    ]
    return (
        cvt(W0c, mdt), [cvt(w, mdt) for w in Wl], cvt(W4c, mdt),
        biases, cvt(np.asarray(b4c).reshape(OUT, 1), np.float32),
    )


def _make_in_maps(inputs):
    x = np.asarray(inputs["text_emb"], dtype=np.float32)
    W0c, Wl, W4c, biases, b4c = _fold_weights(
        np.asarray(inputs["W_text"]), np.asarray(inputs["b_text"]),
        np.asarray(inputs["W_gnn"]), np.asarray(inputs["b_gnn"]),
        np.asarray(inputs["W_out"]), np.asarray(inputs["b_out"]),
        np.asarray(inputs["adjacency"]), np.asarray(inputs["template"]),
    )
    zero_bias = all(np.all(b == 0.0) for b in biases)
    _BUILT.setdefault("zero_bias", zero_bias)
    mdt = _np_mm_dtype()
    in_maps = []
    for c in range(CORES):
        shard = np.ascontiguousarray(x[c * ROWS:(c + 1) * ROWS].T).astype(mdt)
        m = {"xt": shard, "w0": W0c, "w4": W4c, "b4": b4c}
        for i, l in enumerate((1, 2, 3)):
            m[f"w{l}"] = Wl[i]
        if not _BUILT["zero_bias"]:
            for l in range(4):
                m[f"b{l}"] = biases[l]
        in_maps.append(m)
    return in_maps


def kernel(**inputs):
    from concourse.bass_utils import run_bass_kernel_spmd

    in_maps = _make_in_maps(inputs)
    if "nc" not in _BUILT:
        _BUILT["nc"] = _build_bass(repeat=1)
    nc = _BUILT["nc"]
    res = run_bass_kernel_spmd(nc, in_maps, core_ids=list(range(CORES)))
    _BUILT["last_results"] = res
    _BUILT["last_in_maps"] = in_maps

    full = np.empty((B, OUT), dtype=np.float32)
    for c in range(CORES):
        full[c * ROWS:(c + 1) * ROWS] = res.results[c]["out"].T
    return full.reshape(B, 12, 3)



# revision 12
# speedup vs baseline: 1.0195x; 1.0195x over previous
"""MeshGNN Trainium2 kernel (fp8 DoubleRow).

Mathematical reduction: the reference broadcasts the text projection to all 12
mesh vertices, and the row-normalized kNN adjacency has uniform row sums, so
node features stay identical across vertices through every GNN layer.  The
network collapses to a per-row MLP; the 12 per-vertex outputs are 12 copies of
the same 3-vector plus the per-vertex template, so the device computes only
the 3-dim displacement and the host broadcasts (exact math, not an
approximation):

    h   = relu(x @ W0c)               W0c = W_text @ (s*W_gnn[0])  (384,256)
    h   = relu(h @ (s*W_gnn[l]))      l = 1..3
    o3  = h @ W_out                   (B, 3)
    out = template[None] + o3[:, None, :]

Precision strategy: fp8(e4m3) matmuls with DoubleRow (2 rows/cycle on the PE).
Each weight matrix is scaled into e4m3's sweet band (std ~ 4) and each
activation tensor is rescaled to unit rms in the relu stage (scale factors
estimated from a host-side probe of 512 rows, compensated exactly in the next
layer's weights; the final output descale happens on host in f64).

Device schedule (per core, 4096 rows in 8 blocks of N=512):
  - x shard arrives host-transposed as (384, 4096) fp8; one DMA per block
    (layer 0 runs one DoubleRow pair plus one single k-tile matmul).
  - layer-major software pipeline over all 8 blocks; fused two-bank
    [128,2,512] PSUM tiles (both m-halves, bufs=3) so each relu is one op.
  - relu ops (scale*relu in one instruction) alternate between Activation
    and DVE -- the only engines that can read PSUM on TRN2 (GPSIMD cannot).
  - output layer runs transposed: stationary = h3 row-chunk, moving = W4,
    giving [128,4,3] PSUM tiles whose SBUF copies are ~free; one final DMA
    ships (4096, 3) bf16; host descales and broadcasts to (B, 12, 3).
"""

import numpy as np

# ---------------------------------------------------------------- constants
B = 32768
CORES = 8
ROWS = B // CORES            # 4096 rows per core
TD = 384                     # text dim
H = 256                      # hidden
NBLK = 8                     # row blocks per core
N = ROWS // NBLK             # 512 rows per block
MT = H // 128                # 2 m-tiles for hidden outputs

# per-op engine cost estimates (ns) for the load-greedy relu/copy schedule
ENG_COST_RELU = {"A": 1118.0, "D": 1352.0}
ENG_COST_COPY = {"A": 252.0, "D": 208.0}

_BUILT = {}                  # cache: compiled Bass modules + fold results


def _schedule_engines():
    """Greedy engine assignment: relus[l][b] and copies[b] (ACT/DVE only)."""
    load = {"A": 1283.0, "D": 0.0}  # act-table preload
    relus = [[None] * NBLK for _ in range(4)]
    copies = [None] * NBLK
    for l in range(4):
        for b in range(NBLK):
            e = min("AD", key=lambda k: load[k] + ENG_COST_RELU[k])
            load[e] += ENG_COST_RELU[e]
            relus[l][b] = e
            if l == 3:
                e = min("AD", key=lambda k: load[k] + ENG_COST_COPY[k])
                load[e] += ENG_COST_COPY[e]
                copies[b] = e
    return relus, copies


def _build_bass(repeat=1, fake_relu=False, loop_repeat=0, zero_bias=None):
    """Build + compile the per-core Bass program (same NEFF on all cores).

    loop_repeat > 0 wraps the pipeline in a device-side For_i loop executed
    that many times (identical outputs) -- used for dispatch-free HW timing.
    """
    import concourse.mybir as mybir
    import concourse.tile as tile
    from concourse import bacc

    cl = _BUILT["act_scales"]          # [c0..c3] set by _make_in_maps

    f32 = mybir.dt.float32
    bf16 = mybir.dt.bfloat16
    fp8 = mybir.dt.float8e4
    DR = mybir.MatmulPerfMode.DoubleRow
    RELU = mybir.ActivationFunctionType.Relu
    IDENT = mybir.ActivationFunctionType.Identity
    MAX = mybir.AluOpType.max
    MULT = mybir.AluOpType.mult

    RELU_ENG, COPY_ENG = _schedule_engines()

    nc = bacc.Bacc(
        "TRN2",
        target_bir_lowering=False,
        debug=False,
        enable_asserts=False,
        num_devices=CORES,
    )

    xt_d = nc.dram_tensor("xt", (TD, ROWS), fp8, kind="ExternalInput")
    w0_d = nc.dram_tensor("w0", (TD, H), fp8, kind="ExternalInput")
    wh_d = nc.dram_tensor("wh", (3 * H, H), fp8, kind="ExternalInput")
    w4_d = nc.dram_tensor("w4", (H, 3), fp8, kind="ExternalInput")
    out_d = nc.dram_tensor("out", (ROWS, 3), bf16, kind="ExternalOutput")

    # x viewed as (partition, ktile, row)
    xt_v = xt_d.ap().rearrange("(k p) n -> p k n", p=128)
    # out viewed as (partition, block, chunk, coord): row = b*N + j*128 + p
    out_v = out_d.ap().rearrange("(b j p) c -> p b j c", p=128, b=NBLK)

    with tile.TileContext(nc) as tc:
        with (
            tc.tile_pool(name="wp", bufs=1) as wp,
            tc.tile_pool(name="xp", bufs=6) as xp,
            tc.tile_pool(name="hp", bufs=8) as hp,
            tc.tile_pool(name="ob", bufs=1) as obp,
            tc.tile_pool(name="pp", bufs=3, space="PSUM") as pp,
            tc.tile_pool(name="pp4", bufs=2, space="PSUM") as pp4,
        ):
            # ---- weights, loaded once (w0 first on SP so layer 0 starts
            # ~1.5us in; the one-time act-table load hides in the fill)
            # w0: [128, 3, 256]; stationary APs: DR pair + single k-tile
            w0_t = wp.tile([128, 3, H], fp8, tag="w0")
            nc.sync.dma_start(
                w0_t[:], w0_d.ap().rearrange("(i p) m -> p i m", p=128)
            )
            # hidden weights: [128, 6, 256]; layer l (1..3) pair j=(l-1)*2
            wh_t = wp.tile([128, 6, H], fp8, tag="wh")
            nc.gpsimd.dma_start(
                wh_t[:], wh_d.ap().rearrange("(li p) m -> p li m", p=128)
            )
            # output weights: [128, 2, 3]
            w4_t = wp.tile([128, 2, 3], fp8, tag="w4")
            nc.scalar.dma_start(
                w4_t[:], w4_d.ap().rearrange("(i p) m -> p i m", p=128)
            )

            def relu_op(eng, dst, src, c):
                if fake_relu:
                    src = w0_t[:, 0:2, 0:N]
                if eng == "A":
                    nc.scalar.activation(dst, src, RELU, scale=c)
                else:
                    nc.vector.tensor_scalar(dst, src, c, 0.0, MULT, MAX)

            def copy_op(eng, dst, src):
                if eng == "A":
                    nc.scalar.activation(dst, src, IDENT)
                else:
                    nc.vector.tensor_scalar(dst, src, 1.0, None, MULT)

            import contextlib

            loop_cm = (
                tc.For_i(0, loop_repeat, 1) if loop_repeat
                else contextlib.nullcontext()
            )
            with loop_cm:
                for rep in range(repeat):
                    # one SBUF tile accumulates all blocks' final outputs
                    ot = obp.tile([128, NBLK, 4, 3], bf16, tag="ot")

                    xts = {}
                    for b in range(NBLK):
                        xt = xp.tile([128, 3, N], fp8, tag=f"x{b % 6}")
                        nc.sync.dma_start(xt[:], xt_v[:, :, b * N:(b + 1) * N])
                        xts[b] = xt

                    h_prev = {}
                    for l in range(4):
                        for b in range(NBLK):
                            hc = hp.tile(
                                [128, MT, N], fp8,
                                name=f"h{l}{b}", tag=f"h{b % 8}",
                            )
                            ps = pp.tile(
                                [128, MT, N], f32, name="psb", tag="ps"
                            )
                            for m in range(MT):
                                ms = slice(m * 128, (m + 1) * 128)
                                if l == 0:
                                    nc.tensor.matmul(
                                        ps[:, m, :], w0_t[:, 0:2, ms],
                                        xts[b][:, 0:2, :],
                                        start=True, stop=False, perf_mode=DR,
                                    )
                                    nc.tensor.matmul(
                                        ps[:, m, :], w0_t[:, 2, ms],
                                        xts[b][:, 2, :],
                                        start=False, stop=True,
                                    )
                                else:
                                    j = (l - 1) * 2
                                    nc.tensor.matmul(
                                        ps[:, m, :], wh_t[:, j:j + 2, ms],
                                        h_prev[b][:, :, :],
                                        start=True, stop=True, perf_mode=DR,
                                    )
                            relu_op(RELU_ENG[l][b], hc[:], ps[:], cl[l])
                            h_prev[b] = hc
                            if l == 3:
                                # transposed output layer: stationary = h3
                                # row-chunk, moving = W4 -> psum [128, 4, 3]
                                ps4 = pp4.tile([128, 4, 3], f32, tag="ps4")
                                for j in range(4):
                                    nc.tensor.matmul(
                                        ps4[:, j, :],
                                        hc[:, :, j * 128:(j + 1) * 128],
                                        w4_t[:],
                                        start=True, stop=True, perf_mode=DR,
                                    )
                                copy_op(COPY_ENG[b], ot[:, b, :, :], ps4[:])

                    nc.sync.dma_start(out_v[:], ot[:])

    nc.compile()
    return nc


def _fold_weights(x, W_text, b_text, W_gnn, b_gnn, W_out, b_out, adjacency,
                  template):
    """Fold the GNN into a 5-matrix MLP, compute fp8 scale chain from a probe."""
    s_rows = adjacency.astype(np.float64).sum(axis=1)
    if np.ptp(s_rows) > 1e-5:
        raise ValueError("adjacency row sums are not uniform; collapse invalid")
    s = float(s_rows.mean())
    if not (np.all(b_text == 0) and np.all(b_gnn == 0) and np.all(b_out == 0)):
        raise ValueError("nonzero biases unsupported by fp8 kernel")

    W0c = W_text.astype(np.float64) @ (s * W_gnn[0].astype(np.float64))
    Wl = [s * W_gnn[l].astype(np.float64) for l in (1, 2, 3)]
    W4 = W_out.astype(np.float64)

    # probe the true network to get per-layer rms statistics
    xp = x[:512].astype(np.float64)
    z = xp @ W0c
    gamma = []           # 1/rms(h_l)
    h = np.maximum(z, 0.0)
    gamma.append(1.0 / np.sqrt((h ** 2).mean()))
    for l in range(3):
        z = h @ Wl[l]
        h = np.maximum(z, 0.0)
        gamma.append(1.0 / np.sqrt((h ** 2).mean()))

    import concourse.mybir as mybir
    np8 = mybir.dt.np(mybir.dt.float8e4)

    def centered_q(Wb):
        u = 2.0 ** round(np.log2(4.0 / Wb.std()))
        return np.ascontiguousarray((Wb * u).astype(np.float32)).astype(np8), u

    W0q, u0 = centered_q(W0c)
    act_scales = [gamma[0] / u0]
    Whq = []
    for l in range(3):
        Wq, u = centered_q(Wl[l] / gamma[l])
        Whq.append(Wq)
        act_scales.append(gamma[l + 1] / u)
    W4q, u4 = centered_q(W4 / gamma[3])

    return {
        "w0": W0q,
        "wh": np.ascontiguousarray(np.concatenate(Whq, axis=0)),
        "w4": W4q,
        "act_scales": [float(c) for c in act_scales],
        "out_descale": float(1.0 / u4),
    }


def _make_in_maps(inputs):
    x = np.asarray(inputs["text_emb"], dtype=np.float32)
    fold = _fold_weights(
        x, np.asarray(inputs["W_text"]), np.asarray(inputs["b_text"]),
        np.asarray(inputs["W_gnn"]), np.asarray(inputs["b_gnn"]),
        np.asarray(inputs["W_out"]), np.asarray(inputs["b_out"]),
        np.asarray(inputs["adjacency"]), np.asarray(inputs["template"]),
    )
    _BUILT.setdefault("act_scales", fold["act_scales"])
    _BUILT.setdefault("out_descale", fold["out_descale"])
    _BUILT.setdefault("template", np.asarray(inputs["template"], np.float32))

    import concourse.mybir as mybir
    np8 = mybir.dt.np(mybir.dt.float8e4)
    in_maps = []
    for c in range(CORES):
        shard = np.ascontiguousarray(
            x[c * ROWS:(c + 1) * ROWS].T
        ).astype(np8)
        in_maps.append({
            "xt": shard, "w0": fold["w0"], "wh": fold["wh"], "w4": fold["w4"],
        })
    return in_maps


def kernel(**inputs):
    from concourse.bass_utils import run_bass_kernel_spmd

    in_maps = _make_in_maps(inputs)
    if "nc" not in _BUILT:
        _BUILT["nc"] = _build_bass(repeat=1)
    nc = _BUILT["nc"]
    res = run_bass_kernel_spmd(nc, in_maps, core_ids=list(range(CORES)))
    _BUILT["last_results"] = res
    _BUILT["last_in_maps"] = in_maps

    o3 = np.empty((B, 3), dtype=np.float64)
    for c in range(CORES):
        o3[c * ROWS:(c + 1) * ROWS] = res.results[c]["out"].astype(np.float64)
    o3 *= _BUILT["out_descale"]
    out = (
        _BUILT["template"][None, :, :].astype(np.float64)
        + o3[:, None, :]
    ).astype(np.float32)
    return out
